# revision 13
# baseline (speedup 1.0000x reference)
"""Trainium2 Bass kernel for multi-head attention with adaptive span masking.

Computation (per the nn.Module):
    q = (query @ Wq.T) split into B*H rows of size d=64
    attn = softmax((key . q + q @ key_pe) / sqrt(d))
    attn = renormalize(attn * adaptive_span_mask)
    out = (attn . value) merged heads @ Wo.T

Key optimization: the adaptive-span mask zeroes every position before
m0(h) = M - 1 - span[h]*M - RAMP, so those positions contribute exactly
nothing to the masked numerator and Sigma_w.  (They only enter through the
1e-8 * Sigma_e regularizer, whose relative contribution is ~1e-7 --
far below the 2e-2 tolerance.)  The kernel is therefore specialized at
build time to per-head windows W[h] (multiples of 128 positions at the
tail of the M axis) computed on the host from the actual span values;
only K/V rows inside the window are ever read from HBM.  The build is
cached keyed on the span bytes, so a different span triggers a rebuild
(slow host-side compile, still correct).

PE usage notes: per-position matmuls with f1=1 weight columns serialize
on LDWEIGHTS<->MATMUL array conflicts, so (a) the positional scores are
computed in transposed orientation (q stationary once, key_pe streamed
512 columns at a time) and scattered into per-row blocked layout via a
DRAM bounce, and (b) the PV contraction loads 8 weight columns per
LDWEIGHTS and streams 8 value blocks (512 cols) into one [8, 512] PSUM
tile whose row j accumulates w[., 8g+j] . V[., 8g+k, :] in column block
k; the wanted diagonal j==k is extracted with a block-diagonal mask
multiply + per-partition fold + ones-matmul partition sum (every engine
AP must start at partition 0, which rules out direct diagonal reads).

Sharding: batch-parallel across 8 cores.  Core c gets batches [4c, 4c+4)
(all 8 heads) = rows [32c, 32c+32) of key/value; Wq/Wo/key_pe are
replicated.  Each core produces its own [4, 512] output block; the host
concatenates.  No collectives needed.
"""

import math
import os
import sys

import numpy as np

for _p in ("/opt/trn_rl_repo", "/root/.axon_site/_ro/trn_rl_repo"):
    if os.path.isdir(_p) and _p not in sys.path:
        sys.path.insert(0, _p)

import concourse.bass as bass
import concourse.bacc as bacc
import concourse.mybir as mybir
from concourse.bass import ts
from concourse.masks import make_identity
from concourse.tile import TileContext

F32 = mybir.dt.float32

# Problem constants (hardcoded per contest contract)
NHEADS = 8
HEAD_DIM = 64
HID = NHEADS * HEAD_DIM  # 512
B = 32
M = 8192
RAMP = 32.0

N_CORES = 8
BPC = B // N_CORES        # 4 batches per core
NPC = BPC * NHEADS        # 32 (b,h) rows per core
PVG = 4                   # PV weight columns per LDWEIGHTS

_CACHE = {}


def _windows(spans):
    """Per-head active window (multiple of 128, at the tail of M)."""
    W128, S = [], []
    for s in spans:
        s = float(s)
        thr = M - 1.0 - s * M - RAMP  # mask(m) > 0  <=>  m > thr
        m0 = max(0, int(math.floor(thr)) + 1)
        w128 = min(M // 128, max(1, (M - m0 + 127) // 128))
        W128.append(w128)
        S.append(M - 128 * w128)
    return W128, S


def _host_consts(spans):
    """[128, sum(W128)+512] f32: per-head span masks in blocked window
    layout, then the [8, 512] block-diagonal PV extraction mask."""
    W128, S = _windows(spans)
    total = sum(W128)
    hc = np.zeros((128, total + 256), np.float32)
    off = 0
    for h in range(NHEADS):
        W = W128[h]
        p = np.arange(128, dtype=np.float64)[:, None]
        wo = np.arange(W, dtype=np.float64)[None, :]
        m = S[h] + p * W + wo
        mask = np.clip(
            (m - (M - 1) + float(spans[h]) * M) / RAMP + 1.0, 0.0, 1.0
        )
        hc[:, off : off + W] = mask.astype(np.float32)
        off += W
    dm = np.zeros((4, 256), np.float32)
    for j in range(4):
        dm[j, j * 64 : (j + 1) * 64] = 1.0
    hc[0:4, total : total + 256] = dm
    return hc


def build_nc(spans):
    W128, SOFF = _windows(spans)
    WMAX = max(W128)
    SU = min(SOFF)            # union window start (largest head window)
    WU = M - SU               # union window length in positions
    MOFF = [sum(W128[:h]) for h in range(NHEADS)]   # mask col offsets
    DM_OFF = sum(W128)                              # diagmask col offset
    HCW = DM_OFF + 256
    # global row order: big windows first across ALL batches, smallest
    # windows last (minimizes the exposed compute tail after the final DMA)
    HORD = sorted(range(NHEADS), key=lambda h: -W128[h])
    ROWORD = [b * NHEADS + h for h in HORD for b in range(BPC)]

    nc = bacc.Bacc(None, target_bir_lowering=False)
    AF = mybir.ActivationFunctionType
    ALU = mybir.AluOpType
    BF16 = mybir.dt.bfloat16

    q_d = nc.dram_tensor("query", [BPC, HID], F32, kind="ExternalInput")
    k_d = nc.dram_tensor("key", [NPC, M, HEAD_DIM], F32, kind="ExternalInput")
    v_d = nc.dram_tensor("value", [NPC, M, HEAD_DIM], F32, kind="ExternalInput")
    wq_d = nc.dram_tensor("Wq", [HID, HID], F32, kind="ExternalInput")
    wo_d = nc.dram_tensor("Wo", [HID, HID], F32, kind="ExternalInput")
    kpe_d = nc.dram_tensor("key_pe", [HEAD_DIM, M], F32, kind="ExternalInput")
    hc_d = nc.dram_tensor("hconst", [128, HCW], F32, kind="ExternalInput")
    out_d = nc.dram_tensor("out", [BPC, HID], F32, kind="ExternalOutput")

    with TileContext(nc) as tc:
        with (
            tc.tile_pool(name="persist", bufs=1) as persist,
            # main-loop pools created BEFORE setup pools so the kv DMAs get
            # SBUF ranges disjoint from setup tiles (no WAR dep -> kv loads
            # start at t=0, overlapping the whole setup phase)
            tc.tile_pool(name="kv", bufs=4) as kv_pool,
            tc.tile_pool(name="sc", bufs=3) as sc_pool,
            tc.tile_pool(name="fin", bufs=1) as fin_pool,
            tc.tile_pool(name="ps_s", bufs=1, space="PSUM") as ps_s_pool,
            tc.tile_pool(name="ps_o", bufs=1, space="PSUM") as ps_o_pool,
            tc.tile_pool(name="ps_pv", bufs=2, space="PSUM") as ps_pv_pool,
            tc.tile_pool(name="ps_fin", bufs=1, space="PSUM") as ps_fin_pool,
        ):
            identity = persist.tile([128, 128], F32, tag="identity")
            make_identity(nc, identity[:])
            ones_col = persist.tile([128, 1], F32, tag="ones_col")
            nc.vector.memset(ones_col[:], 1.0)

            woT = [persist.tile([128, HID], F32, name=f"woT{j}", tag=f"woT{j}") for j in range(4)]
            q_sb = persist.tile([BPC, HID], F32, tag="q_sb")
            qts = persist.tile([HEAD_DIM, BPC, NHEADS], F32, tag="qts")
            qrep = persist.tile([128, BPC, HID], F32, tag="qrep")
            hconst = persist.tile([128, HCW], F32, tag="hconst")
            nc.scalar.dma_start(out=hconst[:], in_=hc_d[:])
            diagmask = hconst[0:4, DM_OFF : DM_OFF + 256]
            pos_blk = [
                persist.tile(
                    [128, W128[i % NHEADS]], F32, name=f"posb{i}", tag=f"posb{i}"
                )
                for i in range(NPC)
            ]
            ao_sb = persist.tile([1, BPC, HID], F32, tag="ao_sb")

            # ---------------- setup phase A: weight transposes + q ----------
            with (
                tc.tile_pool(name="setupA", bufs=1) as sa,
                tc.tile_pool(name="psA", bufs=2, space="PSUM") as psA,
            ):
                wqT = [sa.tile([128, HID], F32, name=f"wqT{j}", tag=f"wqT{j}") for j in range(4)]
                wq_sb = [sa.tile([128, HID], F32, name=f"wq_sb{i}", tag="wq_sb", bufs=2) for i in range(4)]
                for i in range(4):
                    nc.scalar.dma_start(out=wq_sb[i][:], in_=wq_d[ts(i, 128), :])
                for io in range(4):
                    for jo in range(4):
                        pwt = psA.tile([128, 128], F32, tag="pwt")
                        nc.tensor.matmul(
                            pwt[:], wq_sb[io][:, ts(jo, 128)], identity[:],
                            start=True, stop=True,
                        )
                        nc.vector.tensor_copy(wqT[jo][:, ts(io, 128)], pwt[:])

                query_sb = sa.tile([BPC, HID], F32, tag="query_sb")
                nc.scalar.dma_start(out=query_sb[:], in_=q_d[:])
                qTq = [sa.tile([128, BPC], F32, name=f"qTq{j}", tag=f"qTq{j}") for j in range(4)]
                for jo in range(4):
                    pqt = psA.tile([128, BPC], F32, tag="pwt")
                    nc.tensor.matmul(
                        pqt[:], query_sb[:, ts(jo, 128)], identity[0:BPC, 0:BPC],
                        start=True, stop=True,
                    )
                    nc.vector.tensor_copy(qTq[jo][:], pqt[:])
                # q = query @ Wq.T  ->  [4, 512]
                ps_q = psA.tile([BPC, HID], F32, tag="ps_q", bufs=1)
                for jo in range(4):
                    nc.tensor.matmul(
                        ps_q[:], qTq[jo][:], wqT[jo][:],
                        start=(jo == 0), stop=(jo == 3),
                    )
                nc.vector.tensor_copy(q_sb[:], ps_q[:])
                # qts[d, b, h] = q[b, h*64+d]   (64 partitions)
                for h in range(NHEADS):
                    pqh = psA.tile([HEAD_DIM, BPC], F32, tag="pwt")
                    nc.tensor.matmul(
                        pqh[:], q_sb[:, ts(h, HEAD_DIM)], identity[0:BPC, 0:BPC],
                        start=True, stop=True,
                    )
                    nc.vector.tensor_copy(qts[:, :, h], pqh[:])

            # ---------------- setup phase B: qrep, pos ----------------------
            with (
                tc.tile_pool(name="setupB", bufs=1) as sb,
                tc.tile_pool(name="psB", bufs=2, space="PSUM") as psB,
                tc.tile_pool(name="dramq", bufs=1, space="DRAM") as dq,
            ):
                # q replicated across partitions: qrep[p, b, :] = q[b, :]
                # (bounce via DRAM -- DMA partition-broadcast needs a DRAM src)
                q_dram = dq.tile([BPC, HID], F32, tag="q_dram")
                nc.gpsimd.dma_start(out=q_dram[:], in_=q_sb[:])
                for b in range(BPC):
                    nc.gpsimd.dma_start(
                        out=qrep[:, b, :],
                        in_=q_dram[b : b + 1, :].partition_broadcast(128),
                    )

                # positional scores, transposed orientation over the union
                # window: pos_T[n, j] = sum_d q[n, d] * key_pe[d, SU+j].
                # One stationary LDWEIGHTS (qts, 32 cols), key_pe streamed in
                # 512-column chunks.  Then bounce through DRAM to scatter
                # into per-row blocked layout [128, W128[h]].
                kpe_sb = sb.tile([HEAD_DIM, WU], F32, tag="kpe_sb")
                nc.scalar.dma_start(out=kpe_sb[:], in_=kpe_d[:, SU:])
                pos_T = sb.tile([NPC, WU], F32, tag="pos_T")
                qts_flat = qts[:].rearrange("d b h -> d (b h)")
                for c0 in range(0, WU, 512):
                    cw = min(512, WU - c0)
                    ps_pt = psB.tile([NPC, 512], F32, tag="ps_pt")
                    nc.tensor.matmul(
                        ps_pt[:, :cw], qts_flat, kpe_sb[:, c0 : c0 + cw],
                        start=True, stop=True,
                    )
                    nc.vector.tensor_copy(pos_T[:, c0 : c0 + cw], ps_pt[:, :cw])
                pos_dram = dq.tile([NPC, WU], F32, tag="pos_dram")
                nc.gpsimd.dma_start(out=pos_dram[:], in_=pos_T[:])
                for i in ROWORD:
                    h = i % NHEADS
                    nc.gpsimd.dma_start(
                        out=pos_blk[i][:],
                        in_=pos_dram[i, SOFF[h] - SU :].rearrange(
                            "(p wo) -> p wo", p=128
                        ),
                    )

            # ---------------- setup phase A2: Wo transposes -----------------
            # (after setup B so the PE runs the pos_T matmuls first; copies
            # on the vector engine keep the scalar DMA queue unblocked)
            with (
                tc.tile_pool(name="setupA2", bufs=1) as sa2,
                tc.tile_pool(name="psA2", bufs=2, space="PSUM") as psA2,
            ):
                wo_sb = [sa2.tile([128, HID], F32, name=f"wo_sb{i}", tag="wo_sb", bufs=2) for i in range(4)]
                for i in range(4):
                    nc.scalar.dma_start(out=wo_sb[i][:], in_=wo_d[ts(i, 128), :])
                for io in range(4):
                    for jo in range(4):
                        pwt2 = psA2.tile([128, 128], F32, tag="pwt2")
                        nc.tensor.matmul(
                            pwt2[:], wo_sb[io][:, ts(jo, 128)], identity[:],
                            start=True, stop=True,
                        )
                        nc.vector.tensor_copy(woT[jo][:, ts(io, 128)], pwt2[:])

            # ---------------- main loop over (b, h) rows --------------------
            for i in ROWORD:
                    b, h = divmod(i, NHEADS)
                    W = W128[h]
                    S = SOFF[h]
                    kt = kv_pool.tile([128, WMAX, HEAD_DIM], F32, tag="kt")
                    vt = kv_pool.tile([128, WMAX, HEAD_DIM], F32, tag="vt")
                    nc.sync.dma_start(
                        out=kt[:, :W, :],
                        in_=k_d[i, S:, :].rearrange("(p wo) d -> p wo d", p=128),
                    )
                    nc.scalar.dma_start(
                        out=vt[:, :W, :],
                        in_=v_d[i, S:, :].rearrange("(p wo) d -> p wo d", p=128),
                    )
                    # f32 -> bf16 cast of V on the scalar engine (bf16 PV)
                    vtb = kv_pool.tile([128, WMAX, HEAD_DIM], BF16, tag="vtb")
                    nc.scalar.copy(vtb[:, :W, :], vt[:, :W, :])
                    # content scores: f32 multiply, bf16 prod (reduce reads 2x)
                    prod = sc_pool.tile([128, WMAX, HEAD_DIM], BF16, tag="prod", bufs=1)
                    q_b = (
                        qrep[:, b, ts(h, HEAD_DIM)]
                        .rearrange("p (x d) -> p x d", x=1)
                        .broadcast_to((128, W, HEAD_DIM))
                    )
                    nc.vector.tensor_mul(prod[:, :W, :], kt[:, :W, :], q_b)
                    scores = sc_pool.tile([128, WMAX], F32, tag="scores")
                    nc.vector.reduce_sum(
                        scores[:, :W], prod[:, :W, :], axis=mybir.AxisListType.X
                    )
                    nc.vector.tensor_add(
                        scores[:, :W], scores[:, :W], pos_blk[i][:]
                    )
                    # e = exp(scores / sqrt(d)), Sigma_e fused
                    e_t = sc_pool.tile([128, WMAX], F32, tag="e_t")
                    sums = sc_pool.tile([128, 2], F32, tag="sums")
                    nc.scalar.activation(
                        out=e_t[:, :W], in_=scores[:, :W], func=AF.Exp,
                        scale=float(1.0 / math.sqrt(HEAD_DIM)),
                        accum_out=sums[:, 0:1],
                    )
                    # w = e * mask[h], then Sigma_w
                    w_t = sc_pool.tile([128, WMAX], BF16, tag="w_t")
                    nc.vector.tensor_mul(
                        w_t[:, :W], e_t[:, :W], hconst[:, MOFF[h] : MOFF[h] + W]
                    )
                    nc.vector.reduce_sum(
                        sums[:, 1:2], w_t[:, :W], axis=mybir.AxisListType.X
                    )
                    # partition-reduce both sums: [1, 2] = ones.T @ sums
                    ps_s = ps_s_pool.tile([1, 2], F32, tag="ps_s")
                    nc.tensor.matmul(
                        ps_s[:], ones_col[:], sums[:], start=True, stop=True
                    )
                    sums_sb = sc_pool.tile([1, 2], F32, tag="sums_sb")
                    nc.scalar.copy(sums_sb[:], ps_s[:])
                    # u = Sigma_w + 1e-8 * Sigma_e ; scal = 1/u
                    u_t = sc_pool.tile([1, 1], F32, tag="u_t")
                    nc.scalar.activation(
                        out=u_t[:], in_=sums_sb[:, 0:1], func=AF.Identity,
                        scale=1e-8, bias=sums_sb[:, 1:2],
                    )
                    scal = sc_pool.tile([1, 1], F32, tag="scal")
                    nc.vector.reciprocal(scal[:], u_t[:])
                    # PV: PVG weight columns per LDWEIGHTS, bf16 streams.
                    # A narrower last group still accumulates the correct
                    # diagonal partials (cell [j, j*64+d] only ever sees its
                    # own w[8g+j] * V[8g+j, d] terms).
                    n_j = min(PVG, W)
                    ps_pv = ps_pv_pool.tile([PVG, PVG * HEAD_DIM], F32, tag="ps_pv")
                    ngrp = (W + PVG - 1) // PVG
                    for g in range(ngrp):
                        r = min(PVG, W - g * PVG)
                        nc.tensor.matmul(
                            ps_pv[:r, : r * HEAD_DIM],
                            w_t[:, g * PVG : g * PVG + r],
                            vtb[:, g * PVG : g * PVG + r, :],
                            start=(g == 0),
                            stop=(g == ngrp - 1),
                            skip_group_check=True,
                        )
                    # diagonal extract: mask off-diagonal blocks, fold the 8
                    # column blocks per partition, then ones-matmul the 8
                    # partitions down to one row
                    masked = sc_pool.tile([PVG, PVG * HEAD_DIM], F32, tag="masked")
                    nc.vector.tensor_mul(masked[:], ps_pv[:], diagmask)
                    fhalf = sc_pool.tile([PVG, 2 * HEAD_DIM], F32, tag="fhalf")
                    nc.gpsimd.tensor_tensor(
                        out=fhalf[:], in0=masked[:, 0 : 2 * HEAD_DIM],
                        in1=masked[:, 2 * HEAD_DIM :], op=ALU.add,
                    )
                    folded = sc_pool.tile([PVG, HEAD_DIM], F32, tag="folded")
                    nc.gpsimd.tensor_tensor(
                        out=folded[:], in0=fhalf[:, 0:HEAD_DIM],
                        in1=fhalf[:, HEAD_DIM:], op=ALU.add,
                    )
                    ps_o = ps_o_pool.tile([1, HEAD_DIM], F32, tag="ps_o")
                    nc.tensor.matmul(
                        ps_o[:], ones_col[0:n_j, :], folded[0:n_j, :],
                        start=True, stop=True,
                    )
                    # ao[0, b, h*64:(h+1)*64] = ps_o * scal
                    nc.scalar.activation(
                        out=ao_sb[0:1, b, ts(h, HEAD_DIM)], in_=ps_o[:],
                        func=AF.Copy, scale=scal[:, 0:1],
                    )

            # ---------------- output projection -------------------------
            aoT = []
            for co in range(4):
                ps_t2 = ps_fin_pool.tile([128, BPC], F32, name="ps_t2", tag="ps_fin")
                for b in range(BPC):
                    nc.tensor.matmul(
                        ps_t2[:, b : b + 1],
                        ao_sb[0:1, b, ts(co, 128)],
                        identity[0:1, 0:1],
                        start=True, stop=True,
                    )
                t_sb = fin_pool.tile([128, BPC], F32, name=f"t_sb{co}", tag=f"t_sb{co}")
                nc.scalar.copy(t_sb[:], ps_t2[:])
                aoT.append(t_sb)
            ps_f = ps_fin_pool.tile([BPC, HID], F32, name="ps_f", tag="ps_fin")
            for co in range(4):
                nc.tensor.matmul(
                    ps_f[:], aoT[co][:], woT[co][:],
                    start=(co == 0), stop=(co == 3),
                )
            out_sb = fin_pool.tile([BPC, HID], F32, tag="out_sb")
            nc.scalar.copy(out_sb[:], ps_f[:])
            nc.sync.dma_start(out=out_d[:], in_=out_sb[:])

    nc.compile()
    return nc


def _get_nc(span):
    key = np.ascontiguousarray(np.asarray(span, np.float32)).tobytes()
    if key not in _CACHE:
        spans = tuple(float(x) for x in np.asarray(span, np.float32).ravel())
        _CACHE[key] = build_nc(spans)
    return _CACHE[key]


def _make_in_maps(query, key, value, Wq, Wo, key_pe, span):
    q2 = np.ascontiguousarray(np.asarray(query, np.float32).reshape(B, HID))
    key = np.asarray(key, np.float32)
    value = np.asarray(value, np.float32)
    Wq = np.ascontiguousarray(np.asarray(Wq, np.float32))
    Wo = np.ascontiguousarray(np.asarray(Wo, np.float32))
    key_pe = np.ascontiguousarray(np.asarray(key_pe, np.float32))
    spans = tuple(float(x) for x in np.asarray(span, np.float32).ravel())
    hconst = _host_consts(spans)
    in_maps = []
    for c in range(N_CORES):
        in_maps.append(
            {
                "query": np.ascontiguousarray(q2[c * BPC : (c + 1) * BPC]),
                "key": np.ascontiguousarray(key[c * NPC : (c + 1) * NPC]),
                "value": np.ascontiguousarray(value[c * NPC : (c + 1) * NPC]),
                "Wq": Wq,
                "Wo": Wo,
                "key_pe": key_pe,
                "hconst": hconst,
            }
        )
    return in_maps


def _install_ntff_hook():
    """Shim antenv.axon_hooks with a ctypes NTFF profile hook so
    run_bass_kernel_spmd(trace=True) works in this container."""
    import contextlib
    import ctypes
    import types

    try:
        import antenv.axon_hooks  # noqa: F401

        return
    except ImportError:
        pass
    so_path = "/opt/axon/libaxon_pjrt.so"
    import antenv

    mod = types.ModuleType("antenv.axon_hooks")
    holder = {"hook": None}

    if os.path.exists(so_path):
        lib = ctypes.CDLL(so_path)
        if hasattr(lib, "axon_start_nrt_profile"):
            lib.axon_start_nrt_profile.argtypes = [
                ctypes.POINTER(ctypes.c_int64),
                ctypes.c_size_t,
            ]
            lib.axon_start_nrt_profile.restype = ctypes.c_int64
            lib.axon_stop_nrt_profile.argtypes = [ctypes.c_char_p]
            lib.axon_stop_nrt_profile.restype = ctypes.c_int64

            @contextlib.contextmanager
            def _hook(output_dir, device_ids):
                import jax

                jax.devices()
                if device_ids:
                    ids = (ctypes.c_int64 * len(device_ids))(*device_ids)
                    rc = lib.axon_start_nrt_profile(ids, len(device_ids))
                else:
                    rc = lib.axon_start_nrt_profile(None, 0)
                if rc != 0:
                    raise RuntimeError(f"axon_start_nrt_profile rc={rc}")
                try:
                    yield
                finally:
                    n = lib.axon_stop_nrt_profile(str(output_dir).encode())
                    print(f"profile: {n} file(s) written to {output_dir}")

            holder["hook"] = _hook

    mod.get_axon_ntff_profile_hook = lambda: holder["hook"]
    mod.set_axon_ntff_profile_hook = lambda h: holder.__setitem__("hook", h)
    sys.modules["antenv.axon_hooks"] = mod
    antenv.axon_hooks = mod


def run(query, key, value, Wq, Wo, key_pe, span, trace=False):
    """Run on hardware; returns (output [B,1,HID], BassKernelResults)."""
    from concourse import bass_utils
    from concourse.bass_utils import run_bass_kernel_spmd

    if trace:
        _install_ntff_hook()
        bass_utils.upload_artifacts = lambda tmpdir: f"local:{tmpdir}"
    nc = _get_nc(span)
    in_maps = _make_in_maps(query, key, value, Wq, Wo, key_pe, span)
    res = run_bass_kernel_spmd(nc, in_maps, list(range(N_CORES)), trace=trace)
    out = np.concatenate(
        [np.asarray(res.results[c]["out"]) for c in range(N_CORES)], axis=0
    )
    return out.reshape(B, 1, HID).astype(np.float32), res


def kernel(query, key, value, Wq, Wo, key_pe, span):
    out, _ = run(query, key, value, Wq, Wo, key_pe, span, trace=False)
    return out


# revision 19
# speedup vs baseline: 1.0769x; 1.0769x over previous
"""Trainium2 Bass kernel for multi-head attention with adaptive span masking.

Computation (per the nn.Module):
    q = (query @ Wq.T) split into B*H rows of size d=64
    attn = softmax((key . q + q @ key_pe) / sqrt(d))
    attn = renormalize(attn * adaptive_span_mask)
    out = (attn . value) merged heads @ Wo.T

Key optimization: the adaptive-span mask zeroes every position before
m0(h) = M - 1 - span[h]*M - RAMP, so those positions contribute exactly
nothing to the masked numerator and Sigma_w.  (They only enter through the
1e-8 * Sigma_e regularizer, whose relative contribution is ~1e-7 --
far below the 2e-2 tolerance.)  The kernel is therefore specialized at
build time to per-head windows W[h] (multiples of 128 positions at the
tail of the M axis) computed on the host from the actual span values;
only K/V rows inside the window are ever read from HBM.  The build is
cached keyed on the span bytes, so a different span triggers a rebuild
(slow host-side compile, still correct).

PE usage notes: per-position matmuls with f1=1 weight columns serialize
on LDWEIGHTS<->MATMUL array conflicts, so (a) the positional scores are
computed in transposed orientation (q stationary once, key_pe streamed
512 columns at a time) and scattered into per-row blocked layout via a
DRAM bounce, and (b) the PV contraction loads 8 weight columns per
LDWEIGHTS and streams 8 value blocks (512 cols) into one [8, 512] PSUM
tile whose row j accumulates w[., 8g+j] . V[., 8g+k, :] in column block
k; the wanted diagonal j==k is extracted with a block-diagonal mask
multiply + per-partition fold + ones-matmul partition sum (every engine
AP must start at partition 0, which rules out direct diagonal reads).

Sharding: batch-parallel across 8 cores.  Core c gets batches [4c, 4c+4)
(all 8 heads) = rows [32c, 32c+32) of key/value; Wq/Wo/key_pe are
replicated.  Each core produces its own [4, 512] output block; the host
concatenates.  No collectives needed.
"""

import math
import os
import sys

import numpy as np

for _p in ("/opt/trn_rl_repo", "/root/.axon_site/_ro/trn_rl_repo"):
    if os.path.isdir(_p) and _p not in sys.path:
        sys.path.insert(0, _p)

import concourse.bass as bass
import concourse.bacc as bacc
import concourse.mybir as mybir
from concourse.bass import ts
from concourse.masks import make_identity
from concourse.tile import TileContext

F32 = mybir.dt.float32

# Problem constants (hardcoded per contest contract)
NHEADS = 8
HEAD_DIM = 64
HID = NHEADS * HEAD_DIM  # 512
B = 32
M = 8192
RAMP = 32.0

N_CORES = 8
BPC = B // N_CORES        # 4 batches per core
NPC = BPC * NHEADS        # 32 (b,h) rows per core
PVG = 4                   # PV weight columns per LDWEIGHTS

_CACHE = {}


def _windows(spans):
    """Per-head active window (multiple of 128, at the tail of M)."""
    W128, S = [], []
    for s in spans:
        s = float(s)
        thr = M - 1.0 - s * M - RAMP  # mask(m) > 0  <=>  m > thr
        m0 = max(0, int(math.floor(thr)) + 1)
        w128 = min(M // 128, max(1, (M - m0 + 127) // 128))
        W128.append(w128)
        S.append(M - 128 * w128)
    return W128, S


def _host_consts(spans):
    """[128, sum(W128)+512] f32: per-head span masks in blocked window
    layout, then the [8, 512] block-diagonal PV extraction mask."""
    W128, S = _windows(spans)
    total = sum(W128)
    hc = np.zeros((128, total + 256), np.float32)
    off = 0
    for h in range(NHEADS):
        W = W128[h]
        p = np.arange(128, dtype=np.float64)[:, None]
        wo = np.arange(W, dtype=np.float64)[None, :]
        m = S[h] + p * W + wo
        mask = np.clip(
            (m - (M - 1) + float(spans[h]) * M) / RAMP + 1.0, 0.0, 1.0
        )
        hc[:, off : off + W] = mask.astype(np.float32)
        off += W
    dm = np.zeros((4, 256), np.float32)
    for j in range(4):
        dm[j, j * 64 : (j + 1) * 64] = 1.0
    hc[0:4, total : total + 256] = dm
    return hc


def build_nc(spans):
    W128, SOFF = _windows(spans)
    WMAX = max(W128)
    SU = min(SOFF)            # union window start (largest head window)
    WU = M - SU               # union window length in positions
    MOFF = [sum(W128[:h]) for h in range(NHEADS)]   # mask col offsets
    DM_OFF = sum(W128)                              # diagmask col offset
    HCW = DM_OFF + 256
    # head processing order within each batch: big windows first, the
    # globally-last row gets the smallest window (short pipeline tail)
    HORD = sorted(range(NHEADS), key=lambda h: -W128[h])

    nc = bacc.Bacc(None, target_bir_lowering=False)
    AF = mybir.ActivationFunctionType
    ALU = mybir.AluOpType
    BF16 = mybir.dt.bfloat16

    q_d = nc.dram_tensor("query", [BPC, HID], F32, kind="ExternalInput")
    k_d = nc.dram_tensor("key", [NPC, M, HEAD_DIM], F32, kind="ExternalInput")
    v_d = nc.dram_tensor("value", [NPC, M, HEAD_DIM], F32, kind="ExternalInput")
    wq_d = nc.dram_tensor("Wq", [HID, HID], F32, kind="ExternalInput")
    wo_d = nc.dram_tensor("Wo", [HID, HID], F32, kind="ExternalInput")
    kpe_d = nc.dram_tensor("key_pe", [HEAD_DIM, M], F32, kind="ExternalInput")
    hc_d = nc.dram_tensor("hconst", [128, HCW], F32, kind="ExternalInput")
    out_d = nc.dram_tensor("out", [BPC, HID], F32, kind="ExternalOutput")

    with TileContext(nc) as tc:
        with (
            tc.tile_pool(name="persist", bufs=1) as persist,
            # main-loop pools created BEFORE setup pools so the kv DMAs get
            # SBUF ranges disjoint from setup tiles (no WAR dep -> kv loads
            # start at t=0, overlapping the whole setup phase)
            tc.tile_pool(name="kv", bufs=3) as kv_pool,
            tc.tile_pool(name="sc", bufs=3) as sc_pool,
            tc.tile_pool(name="fin", bufs=1) as fin_pool,
            tc.tile_pool(name="ps_s", bufs=1, space="PSUM") as ps_s_pool,
            tc.tile_pool(name="ps_o", bufs=1, space="PSUM") as ps_o_pool,
            tc.tile_pool(name="ps_pv", bufs=2, space="PSUM") as ps_pv_pool,
            tc.tile_pool(name="ps_fin", bufs=1, space="PSUM") as ps_fin_pool,
        ):
            identity = persist.tile([128, 128], F32, tag="identity")
            make_identity(nc, identity[:])
            ones_col = persist.tile([128, 1], F32, tag="ones_col")
            nc.vector.memset(ones_col[:], 1.0)

            woT = [persist.tile([128, HID], F32, name=f"woT{j}", tag=f"woT{j}") for j in range(4)]
            q_sb = persist.tile([BPC, HID], F32, tag="q_sb")
            qts = persist.tile([HEAD_DIM, BPC, NHEADS], F32, tag="qts")
            qrep = persist.tile([128, BPC, HID], F32, tag="qrep")
            hconst = persist.tile([128, HCW], F32, tag="hconst")
            nc.scalar.dma_start(out=hconst[:], in_=hc_d[:])
            diagmask = hconst[0:4, DM_OFF : DM_OFF + 256]
            pos_blk = [
                persist.tile(
                    [128, W128[i % NHEADS]], F32, name=f"posb{i}", tag=f"posb{i}"
                )
                for i in range(NPC)
            ]
            ao_sb = persist.tile([1, BPC, HID], F32, tag="ao_sb")

            # ---------------- setup phase A: weight transposes + q ----------
            with (
                tc.tile_pool(name="setupA", bufs=1) as sa,
                tc.tile_pool(name="psA", bufs=2, space="PSUM") as psA,
            ):
                wqT = [sa.tile([128, HID], F32, name=f"wqT{j}", tag=f"wqT{j}") for j in range(4)]
                wq_sb = [sa.tile([128, HID], F32, name=f"wq_sb{i}", tag="wq_sb", bufs=2) for i in range(4)]
                for i in range(4):
                    nc.scalar.dma_start(out=wq_sb[i][:], in_=wq_d[ts(i, 128), :])
                for io in range(4):
                    for jo in range(4):
                        pwt = psA.tile([128, 128], F32, tag="pwt")
                        nc.tensor.matmul(
                            pwt[:], wq_sb[io][:, ts(jo, 128)], identity[:],
                            start=True, stop=True,
                        )
                        nc.vector.tensor_copy(wqT[jo][:, ts(io, 128)], pwt[:])

                query_sb = sa.tile([BPC, HID], F32, tag="query_sb")
                nc.scalar.dma_start(out=query_sb[:], in_=q_d[:])
                qTq = [sa.tile([128, BPC], F32, name=f"qTq{j}", tag=f"qTq{j}") for j in range(4)]
                for jo in range(4):
                    pqt = psA.tile([128, BPC], F32, tag="pwt")
                    nc.tensor.matmul(
                        pqt[:], query_sb[:, ts(jo, 128)], identity[0:BPC, 0:BPC],
                        start=True, stop=True,
                    )
                    nc.vector.tensor_copy(qTq[jo][:], pqt[:])
                # q = query @ Wq.T  ->  [4, 512]
                ps_q = psA.tile([BPC, HID], F32, tag="ps_q", bufs=1)
                for jo in range(4):
                    nc.tensor.matmul(
                        ps_q[:], qTq[jo][:], wqT[jo][:],
                        start=(jo == 0), stop=(jo == 3),
                    )
                nc.vector.tensor_copy(q_sb[:], ps_q[:])
                # qts[d, b, h] = q[b, h*64+d]   (64 partitions)
                for h in range(NHEADS):
                    pqh = psA.tile([HEAD_DIM, BPC], F32, tag="pwt")
                    nc.tensor.matmul(
                        pqh[:], q_sb[:, ts(h, HEAD_DIM)], identity[0:BPC, 0:BPC],
                        start=True, stop=True,
                    )
                    nc.vector.tensor_copy(qts[:, :, h], pqh[:])

            # ---------------- setup phase B: qrep, pos ----------------------
            with (
                tc.tile_pool(name="setupB", bufs=1) as sb,
                tc.tile_pool(name="psB", bufs=2, space="PSUM") as psB,
                tc.tile_pool(name="dramq", bufs=1, space="DRAM") as dq,
            ):
                # q replicated across partitions: qrep[p, b, :] = q[b, :]
                # (bounce via DRAM -- DMA partition-broadcast needs a DRAM src)
                q_dram = dq.tile([BPC, HID], F32, tag="q_dram")
                nc.gpsimd.dma_start(out=q_dram[:], in_=q_sb[:])
                for b in range(BPC):
                    nc.gpsimd.dma_start(
                        out=qrep[:, b, :],
                        in_=q_dram[b : b + 1, :].partition_broadcast(128),
                    )

                # positional scores, transposed orientation over the union
                # window: pos_T[n, j] = sum_d q[n, d] * key_pe[d, SU+j].
                # One stationary LDWEIGHTS (qts, 32 cols), key_pe streamed in
                # 512-column chunks.  Then bounce through DRAM to scatter
                # into per-row blocked layout [128, W128[h]].
                kpe_sb = sb.tile([HEAD_DIM, WU], F32, tag="kpe_sb")
                nc.scalar.dma_start(out=kpe_sb[:], in_=kpe_d[:, SU:])
                pos_T = sb.tile([NPC, WU], F32, tag="pos_T")
                qts_flat = qts[:].rearrange("d b h -> d (b h)")
                for c0 in range(0, WU, 512):
                    cw = min(512, WU - c0)
                    ps_pt = psB.tile([NPC, 512], F32, tag="ps_pt")
                    nc.tensor.matmul(
                        ps_pt[:, :cw], qts_flat, kpe_sb[:, c0 : c0 + cw],
                        start=True, stop=True,
                    )
                    nc.vector.tensor_copy(pos_T[:, c0 : c0 + cw], ps_pt[:, :cw])
                pos_dram = dq.tile([NPC, WU], F32, tag="pos_dram")
                nc.gpsimd.dma_start(out=pos_dram[:], in_=pos_T[:])
                for b in range(BPC):
                    for h in HORD:
                        i = b * NHEADS + h
                        nc.gpsimd.dma_start(
                            out=pos_blk[i][:],
                            in_=pos_dram[i, SOFF[h] - SU :].rearrange(
                                "(p wo) -> p wo", p=128
                            ),
                        )

            # ---------------- setup phase A2: Wo transposes -----------------
            # (after setup B so the PE runs the pos_T matmuls first; copies
            # on the vector engine keep the scalar DMA queue unblocked)
            with (
                tc.tile_pool(name="setupA2", bufs=1) as sa2,
                tc.tile_pool(name="psA2", bufs=2, space="PSUM") as psA2,
            ):
                wo_sb = [sa2.tile([128, HID], F32, name=f"wo_sb{i}", tag="wo_sb", bufs=2) for i in range(4)]
                for i in range(4):
                    nc.scalar.dma_start(out=wo_sb[i][:], in_=wo_d[ts(i, 128), :])
                for io in range(4):
                    for jo in range(4):
                        pwt2 = psA2.tile([128, 128], F32, tag="pwt2")
                        nc.tensor.matmul(
                            pwt2[:], wo_sb[io][:, ts(jo, 128)], identity[:],
                            start=True, stop=True,
                        )
                        nc.vector.tensor_copy(woT[jo][:, ts(io, 128)], pwt2[:])

            # ---------------- main loop over (b, h) rows --------------------
            for b in range(BPC):
                for h in HORD:
                    i = b * NHEADS + h
                    W = W128[h]
                    S = SOFF[h]
                    kt = kv_pool.tile([128, WMAX, HEAD_DIM], F32, tag="kt")
                    vt = kv_pool.tile([128, WMAX, HEAD_DIM], F32, tag="vt")
                    nc.sync.dma_start(
                        out=kt[:, :W, :],
                        in_=k_d[i, S:, :].rearrange("(p wo) d -> p wo d", p=128),
                    )
                    nc.scalar.dma_start(
                        out=vt[:, :W, :],
                        in_=v_d[i, S:, :].rearrange("(p wo) d -> p wo d", p=128),
                    )
                    # f32 -> bf16 cast of V on the scalar engine (bf16 PV)
                    vtb = kv_pool.tile([128, WMAX, HEAD_DIM], BF16, tag="vtb")
                    nc.scalar.copy(vtb[:, :W, :], vt[:, :W, :])
                    # content scores: f32 multiply, bf16 prod (reduce reads 2x)
                    prod = sc_pool.tile([128, WMAX, HEAD_DIM], BF16, tag="prod", bufs=1)
                    q_b = (
                        qrep[:, b, ts(h, HEAD_DIM)]
                        .rearrange("p (x d) -> p x d", x=1)
                        .broadcast_to((128, W, HEAD_DIM))
                    )
                    nc.vector.tensor_mul(prod[:, :W, :], kt[:, :W, :], q_b)
                    scores = sc_pool.tile([128, WMAX], F32, tag="scores")
                    nc.vector.reduce_sum(
                        scores[:, :W], prod[:, :W, :], axis=mybir.AxisListType.X
                    )
                    nc.vector.tensor_add(
                        scores[:, :W], scores[:, :W], pos_blk[i][:]
                    )
                    # e = exp(scores / sqrt(d)), Sigma_e fused
                    e_t = sc_pool.tile([128, WMAX], F32, tag="e_t")
                    sums = sc_pool.tile([128, 2], F32, tag="sums")
                    nc.scalar.activation(
                        out=e_t[:, :W], in_=scores[:, :W], func=AF.Exp,
                        scale=float(1.0 / math.sqrt(HEAD_DIM)),
                        accum_out=sums[:, 0:1],
                    )
                    # w = e * mask[h], then Sigma_w
                    w_t = sc_pool.tile([128, WMAX], BF16, tag="w_t")
                    nc.vector.tensor_mul(
                        w_t[:, :W], e_t[:, :W], hconst[:, MOFF[h] : MOFF[h] + W]
                    )
                    nc.vector.reduce_sum(
                        sums[:, 1:2], w_t[:, :W], axis=mybir.AxisListType.X
                    )
                    # partition-reduce both sums: [1, 2] = ones.T @ sums
                    ps_s = ps_s_pool.tile([1, 2], F32, tag="ps_s")
                    nc.tensor.matmul(
                        ps_s[:], ones_col[:], sums[:], start=True, stop=True
                    )
                    sums_sb = sc_pool.tile([1, 2], F32, tag="sums_sb")
                    nc.scalar.copy(sums_sb[:], ps_s[:])
                    # u = Sigma_w + 1e-8 * Sigma_e ; scal = 1/u
                    u_t = sc_pool.tile([1, 1], F32, tag="u_t")
                    nc.scalar.activation(
                        out=u_t[:], in_=sums_sb[:, 0:1], func=AF.Identity,
                        scale=1e-8, bias=sums_sb[:, 1:2],
                    )
                    scal = sc_pool.tile([1, 1], F32, tag="scal")
                    nc.vector.reciprocal(scal[:], u_t[:])
                    # PV: PVG weight columns per LDWEIGHTS, bf16 streams.
                    # A narrower last group still accumulates the correct
                    # diagonal partials (cell [j, j*64+d] only ever sees its
                    # own w[8g+j] * V[8g+j, d] terms).
                    n_j = min(PVG, W)
                    ps_pv = ps_pv_pool.tile([PVG, PVG * HEAD_DIM], F32, tag="ps_pv")
                    ngrp = (W + PVG - 1) // PVG
                    for g in range(ngrp):
                        r = min(PVG, W - g * PVG)
                        nc.tensor.matmul(
                            ps_pv[:r, : r * HEAD_DIM],
                            w_t[:, g * PVG : g * PVG + r],
                            vtb[:, g * PVG : g * PVG + r, :],
                            start=(g == 0),
                            stop=(g == ngrp - 1),
                            skip_group_check=True,
                        )
                    # diagonal extract: mask off-diagonal blocks, fold the 8
                    # column blocks per partition, then ones-matmul the 8
                    # partitions down to one row
                    masked = sc_pool.tile([PVG, PVG * HEAD_DIM], F32, tag="masked")
                    nc.vector.tensor_mul(masked[:], ps_pv[:], diagmask)
                    fhalf = sc_pool.tile([PVG, 2 * HEAD_DIM], F32, tag="fhalf")
                    nc.gpsimd.tensor_tensor(
                        out=fhalf[:], in0=masked[:, 0 : 2 * HEAD_DIM],
                        in1=masked[:, 2 * HEAD_DIM :], op=ALU.add,
                    )
                    folded = sc_pool.tile([PVG, HEAD_DIM], F32, tag="folded")
                    nc.gpsimd.tensor_tensor(
                        out=folded[:], in0=fhalf[:, 0:HEAD_DIM],
                        in1=fhalf[:, HEAD_DIM:], op=ALU.add,
                    )
                    ps_o = ps_o_pool.tile([1, HEAD_DIM], F32, tag="ps_o")
                    nc.tensor.matmul(
                        ps_o[:], ones_col[0:n_j, :], folded[0:n_j, :],
                        start=True, stop=True,
                    )
                    # ao[0, b, h*64:(h+1)*64] = ps_o * scal
                    nc.scalar.activation(
                        out=ao_sb[0:1, b, ts(h, HEAD_DIM)], in_=ps_o[:],
                        func=AF.Copy, scale=scal[:, 0:1],
                    )

            # ---------------- output projection -------------------------
            aoT = []
            for co in range(4):
                ps_t2 = ps_fin_pool.tile([128, BPC], F32, name="ps_t2", tag="ps_fin")
                for b in range(BPC):
                    nc.tensor.matmul(
                        ps_t2[:, b : b + 1],
                        ao_sb[0:1, b, ts(co, 128)],
                        identity[0:1, 0:1],
                        start=True, stop=True,
                    )
                t_sb = fin_pool.tile([128, BPC], F32, name=f"t_sb{co}", tag=f"t_sb{co}")
                nc.scalar.copy(t_sb[:], ps_t2[:])
                aoT.append(t_sb)
            ps_f = ps_fin_pool.tile([BPC, HID], F32, name="ps_f", tag="ps_fin")
            for co in range(4):
                nc.tensor.matmul(
                    ps_f[:], aoT[co][:], woT[co][:],
                    start=(co == 0), stop=(co == 3),
                )
            out_sb = fin_pool.tile([BPC, HID], F32, tag="out_sb")
            nc.scalar.copy(out_sb[:], ps_f[:])
            nc.sync.dma_start(out=out_d[:], in_=out_sb[:])

    nc.compile()
    return nc


def _get_nc(span):
    key = np.ascontiguousarray(np.asarray(span, np.float32)).tobytes()
    if key not in _CACHE:
        spans = tuple(float(x) for x in np.asarray(span, np.float32).ravel())
        _CACHE[key] = build_nc(spans)
    return _CACHE[key]


def _make_in_maps(query, key, value, Wq, Wo, key_pe, span):
    q2 = np.ascontiguousarray(np.asarray(query, np.float32).reshape(B, HID))
    key = np.asarray(key, np.float32)
    value = np.asarray(value, np.float32)
    Wq = np.ascontiguousarray(np.asarray(Wq, np.float32))
    Wo = np.ascontiguousarray(np.asarray(Wo, np.float32))
    key_pe = np.ascontiguousarray(np.asarray(key_pe, np.float32))
    spans = tuple(float(x) for x in np.asarray(span, np.float32).ravel())
    hconst = _host_consts(spans)
    in_maps = []
    for c in range(N_CORES):
        in_maps.append(
            {
                "query": np.ascontiguousarray(q2[c * BPC : (c + 1) * BPC]),
                "key": np.ascontiguousarray(key[c * NPC : (c + 1) * NPC]),
                "value": np.ascontiguousarray(value[c * NPC : (c + 1) * NPC]),
                "Wq": Wq,
                "Wo": Wo,
                "key_pe": key_pe,
                "hconst": hconst,
            }
        )
    return in_maps


def _install_ntff_hook():
    """Shim antenv.axon_hooks with a ctypes NTFF profile hook so
    run_bass_kernel_spmd(trace=True) works in this container."""
    import contextlib
    import ctypes
    import types

    try:
        import antenv.axon_hooks  # noqa: F401

        return
    except ImportError:
        pass
    so_path = "/opt/axon/libaxon_pjrt.so"
    import antenv

    mod = types.ModuleType("antenv.axon_hooks")
    holder = {"hook": None}

    if os.path.exists(so_path):
        lib = ctypes.CDLL(so_path)
        if hasattr(lib, "axon_start_nrt_profile"):
            lib.axon_start_nrt_profile.argtypes = [
                ctypes.POINTER(ctypes.c_int64),
                ctypes.c_size_t,
            ]
            lib.axon_start_nrt_profile.restype = ctypes.c_int64
            lib.axon_stop_nrt_profile.argtypes = [ctypes.c_char_p]
            lib.axon_stop_nrt_profile.restype = ctypes.c_int64

            @contextlib.contextmanager
            def _hook(output_dir, device_ids):
                import jax

                jax.devices()
                if device_ids:
                    ids = (ctypes.c_int64 * len(device_ids))(*device_ids)
                    rc = lib.axon_start_nrt_profile(ids, len(device_ids))
                else:
                    rc = lib.axon_start_nrt_profile(None, 0)
                if rc != 0:
                    raise RuntimeError(f"axon_start_nrt_profile rc={rc}")
                try:
                    yield
                finally:
                    n = lib.axon_stop_nrt_profile(str(output_dir).encode())
                    print(f"profile: {n} file(s) written to {output_dir}")

            holder["hook"] = _hook

    mod.get_axon_ntff_profile_hook = lambda: holder["hook"]
    mod.set_axon_ntff_profile_hook = lambda h: holder.__setitem__("hook", h)
    sys.modules["antenv.axon_hooks"] = mod
    antenv.axon_hooks = mod


def run(query, key, value, Wq, Wo, key_pe, span, trace=False):
    """Run on hardware; returns (output [B,1,HID], BassKernelResults)."""
    from concourse import bass_utils
    from concourse.bass_utils import run_bass_kernel_spmd

    if trace:
        _install_ntff_hook()
        bass_utils.upload_artifacts = lambda tmpdir: f"local:{tmpdir}"
    nc = _get_nc(span)
    in_maps = _make_in_maps(query, key, value, Wq, Wo, key_pe, span)
    res = run_bass_kernel_spmd(nc, in_maps, list(range(N_CORES)), trace=trace)
    out = np.concatenate(
        [np.asarray(res.results[c]["out"]) for c in range(N_CORES)], axis=0
    )
    return out.reshape(B, 1, HID).astype(np.float32), res


def kernel(query, key, value, Wq, Wo, key_pe, span):
    out, _ = run(query, key, value, Wq, Wo, key_pe, span, trace=False)
    return out


# revision 22
# speedup vs baseline: 1.1578x; 1.0751x over previous
"""Trainium2 Bass kernel for multi-head attention with adaptive span masking.

Computation (per the nn.Module):
    q = (query @ Wq.T) split into B*H rows of size d=64
    attn = softmax((key . q + q @ key_pe) / sqrt(d))
    attn = renormalize(attn * adaptive_span_mask)
    out = (attn . value) merged heads @ Wo.T

Key optimization: the adaptive-span mask zeroes every position before
m0(h) = M - 1 - span[h]*M - RAMP, so those positions contribute exactly
nothing to the masked numerator and Sigma_w.  (They only enter through the
1e-8 * Sigma_e regularizer, whose relative contribution is ~1e-7 --
far below the 2e-2 tolerance.)  The kernel is therefore specialized at
build time to per-head windows W[h] (multiples of 128 positions at the
tail of the M axis) computed on the host from the actual span values;
only K/V rows inside the window are ever read from HBM.  The build is
cached keyed on the span bytes, so a different span triggers a rebuild
(slow host-side compile, still correct).

PE usage notes: per-position matmuls with f1=1 weight columns serialize
on LDWEIGHTS<->MATMUL array conflicts, so (a) the positional scores are
computed in transposed orientation (q stationary once, key_pe streamed
512 columns at a time) and scattered into per-row blocked layout via a
DRAM bounce, and (b) the PV contraction loads 8 weight columns per
LDWEIGHTS and streams 8 value blocks (512 cols) into one [8, 512] PSUM
tile whose row j accumulates w[., 8g+j] . V[., 8g+k, :] in column block
k; the wanted diagonal j==k is extracted with a block-diagonal mask
multiply + per-partition fold + ones-matmul partition sum (every engine
AP must start at partition 0, which rules out direct diagonal reads).

Sharding: batch-parallel across 8 cores.  Core c gets batches [4c, 4c+4)
(all 8 heads) = rows [32c, 32c+32) of key/value; Wq/Wo/key_pe are
replicated.  Each core produces its own [4, 512] output block; the host
concatenates.  No collectives needed.
"""

import math
import os
import sys

import numpy as np

for _p in ("/opt/trn_rl_repo", "/root/.axon_site/_ro/trn_rl_repo"):
    if os.path.isdir(_p) and _p not in sys.path:
        sys.path.insert(0, _p)

import concourse.bass as bass
import concourse.bacc as bacc
import concourse.mybir as mybir
from concourse.bass import ts
from concourse.masks import make_identity
from concourse.tile import TileContext

F32 = mybir.dt.float32

# Problem constants (hardcoded per contest contract)
NHEADS = 8
HEAD_DIM = 64
HID = NHEADS * HEAD_DIM  # 512
B = 32
M = 8192
RAMP = 32.0

N_CORES = 8
BPC = B // N_CORES        # 4 batches per core
NPC = BPC * NHEADS        # 32 (b,h) rows per core
PVG = 4                   # PV weight columns per LDWEIGHTS

_CACHE = {}


def _windows(spans):
    """Per-head active window (multiple of 128, at the tail of M)."""
    W128, S = [], []
    for s in spans:
        s = float(s)
        thr = M - 1.0 - s * M - RAMP  # mask(m) > 0  <=>  m > thr
        m0 = max(0, int(math.floor(thr)) + 1)
        w128 = min(M // 128, max(1, (M - m0 + 127) // 128))
        W128.append(w128)
        S.append(M - 128 * w128)
    return W128, S


def _host_consts(spans):
    """[128, sum(W128)+512] f32: per-head span masks in blocked window
    layout, then the [8, 512] block-diagonal PV extraction mask."""
    W128, S = _windows(spans)
    total = sum(W128)
    hc = np.zeros((128, total + 256), np.float32)
    off = 0
    for h in range(NHEADS):
        W = W128[h]
        p = np.arange(128, dtype=np.float64)[:, None]
        wo = np.arange(W, dtype=np.float64)[None, :]
        m = S[h] + p * W + wo
        mask = np.clip(
            (m - (M - 1) + float(spans[h]) * M) / RAMP + 1.0, 0.0, 1.0
        )
        hc[:, off : off + W] = mask.astype(np.float32)
        off += W
    dm = np.zeros((4, 256), np.float32)
    for j in range(4):
        dm[j, j * 64 : (j + 1) * 64] = 1.0
    hc[0:4, total : total + 256] = dm
    return hc


def build_nc(spans):
    W128, SOFF = _windows(spans)
    WMAX = max(W128)
    SU = min(SOFF)            # union window start (largest head window)
    WU = M - SU               # union window length in positions
    MOFF = [sum(W128[:h]) for h in range(NHEADS)]   # mask col offsets
    DM_OFF = sum(W128)                              # diagmask col offset
    HCW = DM_OFF + 256
    # head processing order within each batch: big windows first, the
    # globally-last row gets the smallest window (short pipeline tail)
    HORD = sorted(range(NHEADS), key=lambda h: -W128[h])
    ROWORD = [b * NHEADS + h for b in range(BPC) for h in HORD]

    nc = bacc.Bacc(None, target_bir_lowering=False)
    AF = mybir.ActivationFunctionType
    ALU = mybir.AluOpType
    BF16 = mybir.dt.bfloat16

    q_d = nc.dram_tensor("query", [BPC, HID], F32, kind="ExternalInput")
    k_d = nc.dram_tensor("key", [NPC, M, HEAD_DIM], F32, kind="ExternalInput")
    v_d = nc.dram_tensor("value", [NPC, M, HEAD_DIM], F32, kind="ExternalInput")
    wq_d = nc.dram_tensor("Wq", [HID, HID], F32, kind="ExternalInput")
    wo_d = nc.dram_tensor("Wo", [HID, HID], F32, kind="ExternalInput")
    kpe_d = nc.dram_tensor("key_pe", [HEAD_DIM, M], F32, kind="ExternalInput")
    hc_d = nc.dram_tensor("hconst", [128, HCW], F32, kind="ExternalInput")
    out_d = nc.dram_tensor("out", [BPC, HID], F32, kind="ExternalOutput")

    with TileContext(nc) as tc:
        with (
            tc.tile_pool(name="persist", bufs=1) as persist,
            # main-loop pools created BEFORE setup pools so the kv DMAs get
            # SBUF ranges disjoint from setup tiles (no WAR dep -> kv loads
            # start at t=0, overlapping the whole setup phase)
            tc.tile_pool(name="kv", bufs=3) as kv_pool,
            tc.tile_pool(name="sc", bufs=3) as sc_pool,
            tc.tile_pool(name="fin", bufs=1) as fin_pool,
            tc.tile_pool(name="ps_s", bufs=1, space="PSUM") as ps_s_pool,
            tc.tile_pool(name="ps_o", bufs=1, space="PSUM") as ps_o_pool,
            tc.tile_pool(name="ps_pv", bufs=2, space="PSUM") as ps_pv_pool,
            tc.tile_pool(name="ps_fin", bufs=1, space="PSUM") as ps_fin_pool,
        ):
            identity = persist.tile([128, 128], F32, tag="identity")
            make_identity(nc, identity[:])
            ones_col = persist.tile([128, 1], F32, tag="ones_col")
            nc.vector.memset(ones_col[:], 1.0)

            woT = [persist.tile([128, HID], F32, name=f"woT{j}", tag=f"woT{j}") for j in range(4)]
            q_sb = persist.tile([BPC, HID], F32, tag="q_sb")
            qts = persist.tile([HEAD_DIM, BPC, NHEADS], F32, tag="qts")
            qrep = persist.tile([128, BPC, HID], F32, tag="qrep")
            hconst = persist.tile([128, HCW], F32, tag="hconst")
            nc.scalar.dma_start(out=hconst[:], in_=hc_d[:])
            diagmask = hconst[0:4, DM_OFF : DM_OFF + 256]
            pos_blk = [
                persist.tile(
                    [128, BPC, W128[h]], F32, name=f"posb{h}", tag=f"posb{h}"
                )
                for h in range(NHEADS)
            ]
            ao_sb = persist.tile([1, BPC, HID], F32, tag="ao_sb")

            # ---------------- setup phase A: weight transposes + q ----------
            with (
                tc.tile_pool(name="setupA", bufs=1) as sa,
                tc.tile_pool(name="psA", bufs=2, space="PSUM") as psA,
            ):
                wqT = [sa.tile([128, HID], F32, name=f"wqT{j}", tag=f"wqT{j}") for j in range(4)]
                wq_sb = [sa.tile([128, HID], F32, name=f"wq_sb{i}", tag="wq_sb", bufs=2) for i in range(4)]
                for i in range(4):
                    nc.scalar.dma_start(out=wq_sb[i][:], in_=wq_d[ts(i, 128), :])
                for io in range(4):
                    for jo in range(4):
                        pwt = psA.tile([128, 128], F32, tag="pwt")
                        nc.tensor.matmul(
                            pwt[:], wq_sb[io][:, ts(jo, 128)], identity[:],
                            start=True, stop=True,
                        )
                        nc.vector.tensor_copy(wqT[jo][:, ts(io, 128)], pwt[:])

                query_sb = sa.tile([BPC, HID], F32, tag="query_sb")
                nc.scalar.dma_start(out=query_sb[:], in_=q_d[:])
                qTq = [sa.tile([128, BPC], F32, name=f"qTq{j}", tag=f"qTq{j}") for j in range(4)]
                for jo in range(4):
                    pqt = psA.tile([128, BPC], F32, tag="pwt")
                    nc.tensor.matmul(
                        pqt[:], query_sb[:, ts(jo, 128)], identity[0:BPC, 0:BPC],
                        start=True, stop=True,
                    )
                    nc.vector.tensor_copy(qTq[jo][:], pqt[:])
                # q = query @ Wq.T  ->  [4, 512]
                ps_q = psA.tile([BPC, HID], F32, tag="ps_q", bufs=1)
                for jo in range(4):
                    nc.tensor.matmul(
                        ps_q[:], qTq[jo][:], wqT[jo][:],
                        start=(jo == 0), stop=(jo == 3),
                    )
                nc.vector.tensor_copy(q_sb[:], ps_q[:])
                # qts[d, b, h] = q[b, h*64+d]   (64 partitions)
                for h in range(NHEADS):
                    pqh = psA.tile([HEAD_DIM, BPC], F32, tag="pwt")
                    nc.tensor.matmul(
                        pqh[:], q_sb[:, ts(h, HEAD_DIM)], identity[0:BPC, 0:BPC],
                        start=True, stop=True,
                    )
                    nc.vector.tensor_copy(qts[:, :, h], pqh[:])

            # ---------------- setup phase B: qrep, pos ----------------------
            with (
                tc.tile_pool(name="setupB", bufs=1) as sb,
                tc.tile_pool(name="psB", bufs=2, space="PSUM") as psB,
                tc.tile_pool(name="dramq", bufs=1, space="DRAM") as dq,
            ):
                # q replicated across partitions: qrep[p, b, :] = q[b, :]
                # (bounce via DRAM -- DMA partition-broadcast needs a DRAM src)
                q_dram = dq.tile([BPC, HID], F32, tag="q_dram")
                nc.gpsimd.dma_start(out=q_dram[:], in_=q_sb[:])
                for b in range(BPC):
                    nc.gpsimd.dma_start(
                        out=qrep[:, b, :],
                        in_=q_dram[b : b + 1, :].partition_broadcast(128),
                    )

                # positional scores, transposed orientation over the union
                # window: pos_T[n, j] = sum_d q[n, d] * key_pe[d, SU+j].
                # One stationary LDWEIGHTS (qts, 32 cols), key_pe streamed in
                # 512-column chunks.  Then bounce through DRAM to scatter
                # into per-row blocked layout [128, W128[h]].
                kpe_sb = sb.tile([HEAD_DIM, WU], F32, tag="kpe_sb")
                nc.scalar.dma_start(out=kpe_sb[:], in_=kpe_d[:, SU:])
                pos_T = sb.tile([NPC, WU], F32, tag="pos_T")
                qts_flat = qts[:].rearrange("d b h -> d (b h)")
                for c0 in range(0, WU, 512):
                    cw = min(512, WU - c0)
                    ps_pt = psB.tile([NPC, 512], F32, tag="ps_pt")
                    nc.tensor.matmul(
                        ps_pt[:, :cw], qts_flat, kpe_sb[:, c0 : c0 + cw],
                        start=True, stop=True,
                    )
                    nc.vector.tensor_copy(pos_T[:, c0 : c0 + cw], ps_pt[:, :cw])
                pos_dram = dq.tile([NPC, WU], F32, tag="pos_dram")
                nc.gpsimd.dma_start(out=pos_dram[:], in_=pos_T[:])
                pos_dram_v = pos_dram[:].rearrange(
                    "(b hh) wu -> b hh wu", hh=NHEADS
                )
                for h in HORD:
                    nc.gpsimd.dma_start(
                        out=pos_blk[h][:],
                        in_=pos_dram_v[:, h, SOFF[h] - SU :].rearrange(
                            "b (p wo) -> p b wo", p=128
                        ),
                    )

            # ---------------- setup phase A2: Wo transposes -----------------
            # (after setup B so the PE runs the pos_T matmuls first; copies
            # on the vector engine keep the scalar DMA queue unblocked)
            with (
                tc.tile_pool(name="setupA2", bufs=1) as sa2,
                tc.tile_pool(name="psA2", bufs=2, space="PSUM") as psA2,
            ):
                wo_sb = [sa2.tile([128, HID], F32, name=f"wo_sb{i}", tag="wo_sb", bufs=2) for i in range(4)]
                for i in range(4):
                    nc.scalar.dma_start(out=wo_sb[i][:], in_=wo_d[ts(i, 128), :])
                for io in range(4):
                    for jo in range(4):
                        pwt2 = psA2.tile([128, 128], F32, tag="pwt2")
                        nc.tensor.matmul(
                            pwt2[:], wo_sb[io][:, ts(jo, 128)], identity[:],
                            start=True, stop=True,
                        )
                        nc.vector.tensor_copy(woT[jo][:, ts(io, 128)], pwt2[:])

            # ---------------- main loop over (b, h) rows --------------------
            # K/V DMA issues are software-pipelined two rows ahead so the
            # V-load issue on the scalar engine never sits behind the
            # previous row's cast/exp work (keeps both HWDGE queues fed).
            kv_tiles = {}

            def issue_kv(r):
                ii = ROWORD[r]
                hh = ii % NHEADS
                Wr = W128[hh]
                Sr = SOFF[hh]
                ktile = kv_pool.tile([128, WMAX, HEAD_DIM], F32, tag="kt")
                vtile = kv_pool.tile([128, WMAX, HEAD_DIM], F32, tag="vt")
                nc.sync.dma_start(
                    out=ktile[:, :Wr, :],
                    in_=k_d[ii, Sr:, :].rearrange("(p wo) d -> p wo d", p=128),
                )
                nc.scalar.dma_start(
                    out=vtile[:, :Wr, :],
                    in_=v_d[ii, Sr:, :].rearrange("(p wo) d -> p wo d", p=128),
                )
                kv_tiles[r] = (ktile, vtile)

            issue_kv(0)
            issue_kv(1)
            for r in range(NPC):
                    i = ROWORD[r]
                    b, h = divmod(i, NHEADS)
                    W = W128[h]
                    S = SOFF[h]
                    if r + 2 < NPC:
                        issue_kv(r + 2)
                    kt, vt = kv_tiles.pop(r)
                    # f32 -> bf16 cast of V on the scalar engine (bf16 PV)
                    vtb = kv_pool.tile([128, WMAX, HEAD_DIM], BF16, tag="vtb")
                    nc.scalar.copy(vtb[:, :W, :], vt[:, :W, :])
                    # content scores: f32 multiply, bf16 prod (reduce reads 2x)
                    prod = sc_pool.tile([128, WMAX, HEAD_DIM], BF16, tag="prod", bufs=1)
                    q_b = (
                        qrep[:, b, ts(h, HEAD_DIM)]
                        .rearrange("p (x d) -> p x d", x=1)
                        .broadcast_to((128, W, HEAD_DIM))
                    )
                    nc.vector.tensor_mul(prod[:, :W, :], kt[:, :W, :], q_b)
                    scores = sc_pool.tile([128, WMAX], F32, tag="scores")
                    nc.vector.reduce_sum(
                        scores[:, :W], prod[:, :W, :], axis=mybir.AxisListType.X
                    )
                    nc.vector.tensor_add(
                        scores[:, :W], scores[:, :W], pos_blk[h][:, b, :]
                    )
                    # e = exp(scores / sqrt(d)), Sigma_e fused
                    e_t = sc_pool.tile([128, WMAX], F32, tag="e_t")
                    sums = sc_pool.tile([128, 2], F32, tag="sums")
                    nc.scalar.activation(
                        out=e_t[:, :W], in_=scores[:, :W], func=AF.Exp,
                        scale=float(1.0 / math.sqrt(HEAD_DIM)),
                        accum_out=sums[:, 0:1],
                    )
                    # w = e * mask[h], then Sigma_w
                    w_t = sc_pool.tile([128, WMAX], BF16, tag="w_t")
                    nc.vector.tensor_mul(
                        w_t[:, :W], e_t[:, :W], hconst[:, MOFF[h] : MOFF[h] + W]
                    )
                    nc.vector.reduce_sum(
                        sums[:, 1:2], w_t[:, :W], axis=mybir.AxisListType.X
                    )
                    # partition-reduce both sums: [1, 2] = ones.T @ sums
                    ps_s = ps_s_pool.tile([1, 2], F32, tag="ps_s")
                    nc.tensor.matmul(
                        ps_s[:], ones_col[:], sums[:], start=True, stop=True
                    )
                    sums_sb = sc_pool.tile([1, 2], F32, tag="sums_sb")
                    nc.scalar.copy(sums_sb[:], ps_s[:])
                    # u = Sigma_w + 1e-8 * Sigma_e ; scal = 1/u
                    u_t = sc_pool.tile([1, 1], F32, tag="u_t")
                    nc.scalar.activation(
                        out=u_t[:], in_=sums_sb[:, 0:1], func=AF.Identity,
                        scale=1e-8, bias=sums_sb[:, 1:2],
                    )
                    scal = sc_pool.tile([1, 1], F32, tag="scal")
                    nc.vector.reciprocal(scal[:], u_t[:])
                    # PV: PVG weight columns per LDWEIGHTS, bf16 streams.
                    # A narrower last group still accumulates the correct
                    # diagonal partials (cell [j, j*64+d] only ever sees its
                    # own w[8g+j] * V[8g+j, d] terms).
                    n_j = min(PVG, W)
                    ps_pv = ps_pv_pool.tile([PVG, PVG * HEAD_DIM], F32, tag="ps_pv")
                    ngrp = (W + PVG - 1) // PVG
                    for g in range(ngrp):
                        r = min(PVG, W - g * PVG)
                        nc.tensor.matmul(
                            ps_pv[:r, : r * HEAD_DIM],
                            w_t[:, g * PVG : g * PVG + r],
                            vtb[:, g * PVG : g * PVG + r, :],
                            start=(g == 0),
                            stop=(g == ngrp - 1),
                            skip_group_check=True,
                        )
                    # diagonal extract: mask off-diagonal blocks, fold the 8
                    # column blocks per partition, then ones-matmul the 8
                    # partitions down to one row
                    masked = sc_pool.tile([PVG, PVG * HEAD_DIM], F32, tag="masked")
                    nc.vector.tensor_mul(masked[:], ps_pv[:], diagmask)
                    fhalf = sc_pool.tile([PVG, 2 * HEAD_DIM], F32, tag="fhalf")
                    nc.gpsimd.tensor_tensor(
                        out=fhalf[:], in0=masked[:, 0 : 2 * HEAD_DIM],
                        in1=masked[:, 2 * HEAD_DIM :], op=ALU.add,
                    )
                    folded = sc_pool.tile([PVG, HEAD_DIM], F32, tag="folded")
                    nc.gpsimd.tensor_tensor(
                        out=folded[:], in0=fhalf[:, 0:HEAD_DIM],
                        in1=fhalf[:, HEAD_DIM:], op=ALU.add,
                    )
                    ps_o = ps_o_pool.tile([1, HEAD_DIM], F32, tag="ps_o")
                    nc.tensor.matmul(
                        ps_o[:], ones_col[0:n_j, :], folded[0:n_j, :],
                        start=True, stop=True,
                    )
                    # ao[0, b, h*64:(h+1)*64] = ps_o * scal
                    nc.scalar.activation(
                        out=ao_sb[0:1, b, ts(h, HEAD_DIM)], in_=ps_o[:],
                        func=AF.Copy, scale=scal[:, 0:1],
                    )

            # ---------------- output projection -------------------------
            aoT = []
            for co in range(4):
                ps_t2 = ps_fin_pool.tile([128, BPC], F32, name="ps_t2", tag="ps_fin")
                for b in range(BPC):
                    nc.tensor.matmul(
                        ps_t2[:, b : b + 1],
                        ao_sb[0:1, b, ts(co, 128)],
                        identity[0:1, 0:1],
                        start=True, stop=True,
                    )
                t_sb = fin_pool.tile([128, BPC], F32, name=f"t_sb{co}", tag=f"t_sb{co}")
                nc.scalar.copy(t_sb[:], ps_t2[:])
                aoT.append(t_sb)
            ps_f = ps_fin_pool.tile([BPC, HID], F32, name="ps_f", tag="ps_fin")
            for co in range(4):
                nc.tensor.matmul(
                    ps_f[:], aoT[co][:], woT[co][:],
                    start=(co == 0), stop=(co == 3),
                )
            out_sb = fin_pool.tile([BPC, HID], F32, tag="out_sb")
            nc.scalar.copy(out_sb[:], ps_f[:])
            nc.sync.dma_start(out=out_d[:], in_=out_sb[:])

    nc.compile()
    return nc


def _get_nc(span):
    key = np.ascontiguousarray(np.asarray(span, np.float32)).tobytes()
    if key not in _CACHE:
        spans = tuple(float(x) for x in np.asarray(span, np.float32).ravel())
        _CACHE[key] = build_nc(spans)
    return _CACHE[key]


def _make_in_maps(query, key, value, Wq, Wo, key_pe, span):
    q2 = np.ascontiguousarray(np.asarray(query, np.float32).reshape(B, HID))
    key = np.asarray(key, np.float32)
    value = np.asarray(value, np.float32)
    Wq = np.ascontiguousarray(np.asarray(Wq, np.float32))
    Wo = np.ascontiguousarray(np.asarray(Wo, np.float32))
    key_pe = np.ascontiguousarray(np.asarray(key_pe, np.float32))
    spans = tuple(float(x) for x in np.asarray(span, np.float32).ravel())
    hconst = _host_consts(spans)
    in_maps = []
    for c in range(N_CORES):
        in_maps.append(
            {
                "query": np.ascontiguousarray(q2[c * BPC : (c + 1) * BPC]),
                "key": np.ascontiguousarray(key[c * NPC : (c + 1) * NPC]),
                "value": np.ascontiguousarray(value[c * NPC : (c + 1) * NPC]),
                "Wq": Wq,
                "Wo": Wo,
                "key_pe": key_pe,
                "hconst": hconst,
            }
        )
    return in_maps


def _install_ntff_hook():
    """Shim antenv.axon_hooks with a ctypes NTFF profile hook so
    run_bass_kernel_spmd(trace=True) works in this container."""
    import contextlib
    import ctypes
    import types

    try:
        import antenv.axon_hooks  # noqa: F401

        return
    except ImportError:
        pass
    so_path = "/opt/axon/libaxon_pjrt.so"
    import antenv

    mod = types.ModuleType("antenv.axon_hooks")
    holder = {"hook": None}

    if os.path.exists(so_path):
        lib = ctypes.CDLL(so_path)
        if hasattr(lib, "axon_start_nrt_profile"):
            lib.axon_start_nrt_profile.argtypes = [
                ctypes.POINTER(ctypes.c_int64),
                ctypes.c_size_t,
            ]
            lib.axon_start_nrt_profile.restype = ctypes.c_int64
            lib.axon_stop_nrt_profile.argtypes = [ctypes.c_char_p]
            lib.axon_stop_nrt_profile.restype = ctypes.c_int64

            @contextlib.contextmanager
            def _hook(output_dir, device_ids):
                import jax

                jax.devices()
                if device_ids:
                    ids = (ctypes.c_int64 * len(device_ids))(*device_ids)
                    rc = lib.axon_start_nrt_profile(ids, len(device_ids))
                else:
                    rc = lib.axon_start_nrt_profile(None, 0)
                if rc != 0:
                    raise RuntimeError(f"axon_start_nrt_profile rc={rc}")
                try:
                    yield
                finally:
                    n = lib.axon_stop_nrt_profile(str(output_dir).encode())
                    print(f"profile: {n} file(s) written to {output_dir}")

            holder["hook"] = _hook

    mod.get_axon_ntff_profile_hook = lambda: holder["hook"]
    mod.set_axon_ntff_profile_hook = lambda h: holder.__setitem__("hook", h)
    sys.modules["antenv.axon_hooks"] = mod
    antenv.axon_hooks = mod


def run(query, key, value, Wq, Wo, key_pe, span, trace=False):
    """Run on hardware; returns (output [B,1,HID], BassKernelResults)."""
    from concourse import bass_utils
    from concourse.bass_utils import run_bass_kernel_spmd

    if trace:
        _install_ntff_hook()
        bass_utils.upload_artifacts = lambda tmpdir: f"local:{tmpdir}"
    nc = _get_nc(span)
    in_maps = _make_in_maps(query, key, value, Wq, Wo, key_pe, span)
    res = run_bass_kernel_spmd(nc, in_maps, list(range(N_CORES)), trace=trace)
    out = np.concatenate(
        [np.asarray(res.results[c]["out"]) for c in range(N_CORES)], axis=0
    )
    return out.reshape(B, 1, HID).astype(np.float32), res


def kernel(query, key, value, Wq, Wo, key_pe, span):
    out, _ = run(query, key, value, Wq, Wo, key_pe, span, trace=False)
    return out
